# revision 15
# baseline (speedup 1.0000x reference)
"""Multi-head attention with "restricted softmax" on 8 TRN2 NeuronCores.

Reference computation (per head):
    score = Q @ K.T / sqrt(D)                       # [S, S]
    attn  = exp(score) / (1 + sum_k exp(score))     # restricted softmax
            (mathematically identical to the max-clamped reference form)
    out   = attn @ V                                # [S, D]

Full problem: B=2, H=16, S=2048, D=64  ->  32 heads, 4 heads per core.

Per-core kernel strategy (no communication needed):
  - Scores computed TRANSPOSED (S^T[k, q]) so softmax's k-reduction sits on
    the PSUM partition axis where the PE performs it for free: PV uses
    lhsT=[V | 1] so the extra output row is sum_k exp = the denominator.
  - All matmul operands are fp16 (PE streams 1 col/cycle vs fp32's slower
    modes) with fp32 PSUM accumulation.
  - The scores contraction (d=64) is ZERO-PADDED to K=128: half-height
    weights block LDWEIGHTS pipelining (measured 427 ns vs 216 ns per
    512-col matmul); zeros in kT rows 64-127 make qT's bottom half inert.
  - exp is fused with the 1/sqrt(D) scale on the ScalarEngine while
    evicting scores PSUM->SBUF (the ScalarEngine is the co-bottleneck).
  - Emission is software-pipelined: scores(k+1) before PV(k); transpose
    prologues are spread into earlier passes' PE slack.
"""

import os

import numpy as np

import concourse.bass as bass  # noqa: F401  (bass must import before tile)
import concourse.mybir as mybir
import concourse.tile as tile
from concourse import bacc
from concourse.bass_utils import run_bass_kernel_spmd
from concourse.masks import make_identity

B, H, S, D = 2, 16, 2048, 64
N_CORES = 8
HPC = (B * H) // N_CORES  # heads per core = 4

F32 = mybir.dt.float32
F16 = mybir.dt.float16
EXP = mybir.ActivationFunctionType.Exp

SCALE = 1.0 / 8.0  # 1/sqrt(D)
NQ = S // 128      # 16 tiles of 128 along both q and k
QH = 1024          # q-half width processed per pass
NB = QH // 512     # 512-wide matmuls per scores tile


class _HeadInputs:
    """Per-head staged inputs: fp16 Q^T/K^T [128, S] (rows 0..63 data,
    rows 64..127 exact zeros so the scores matmul contracts over K=128 —
    half-height weights block LDWEIGHTS pipelining) and [V|1].

    The transposes run on the DMA X-bar (fp16, [128,128] tiles): zero PE
    cost, and the d-padding produces the zero rows for free."""

    def __init__(self, ctx, h):
        self.ctx = ctx
        self.h = h

    def start_dma(self):
        nc, pools, h = self.ctx["nc"], self.ctx, self.h
        head_pool = pools["head_pool"]
        q_nat = head_pool.tile([128, NQ, D], F32, tag="q_nat", name=f"q_nat{h}")
        nc.sync.dma_start(q_nat[:], pools["q_dram"][h].rearrange("(n p) d -> p n d", p=128))
        k_nat = head_pool.tile([128, NQ, D], F32, tag="k_nat", name=f"k_nat{h}")
        nc.sync.dma_start(k_nat[:], pools["k_dram"][h].rearrange("(n p) d -> p n d", p=128))
        v_nat = head_pool.tile([128, NQ, D], F32, tag="v_nat", name=f"v_nat{h}")
        nc.sync.dma_start(v_nat[:], pools["v_dram"][h].rearrange("(n p) d -> p n d", p=128))

        # fp16 staging with d padded 64->128; pad columns stay zero across
        # slot reuse (each head only rewrites cols 0..63), cleared on the
        # first two uses (one per pool slot).
        q16 = head_pool.tile([128, NQ, 128], F16, tag="q16", name=f"q16_{h}")
        k16 = head_pool.tile([128, NQ, 128], F16, tag="k16", name=f"k16_{h}")
        if h < 2:
            nc.vector.memset(q16[:, :, D:], 0.0)
            nc.vector.memset(k16[:, :, D:], 0.0)
        nc.vector.tensor_copy(q16[:, :, :D], q_nat[:])
        nc.vector.tensor_copy(k16[:, :, :D], k_nat[:])
        v1 = head_pool.tile([128, NQ, D + 1], F16, tag="v1", name=f"v1_{h}")
        nc.vector.tensor_copy(
            v1[:, :, D:].rearrange("p n one -> p (n one)"), pools["ones"][:]
        )
        nc.vector.tensor_copy(v1[:, :, :D], v_nat[:])
        self.q16, self.k16, self.v1 = q16, k16, v1

        qkt_pool = pools["qkt_pool"]
        self.qT = qkt_pool.tile([128, S], F16, tag="qT", name=f"qT{h}")
        self.kT = qkt_pool.tile([128, S], F16, tag="kT", name=f"kT{h}")
        # X-bar transposes: [128 q, 128 d_pad] -> [128 d_pad, 128 q] per tile
        for n in range(NQ):
            nc.sync.dma_start_transpose(
                self.qT[:, n * 128:(n + 1) * 128], self.q16[:, n, :]
            )
            nc.sync.dma_start_transpose(
                self.kT[:, n * 128:(n + 1) * 128], self.k16[:, n, :]
            )


def _attention(tc):
    nc = tc.nc
    q_dram = nc.dram_tensor("query", [HPC, S, D], F32, kind="ExternalInput").ap()
    k_dram = nc.dram_tensor("key", [HPC, S, D], F32, kind="ExternalInput").ap()
    v_dram = nc.dram_tensor("value", [HPC, S, D], F32, kind="ExternalInput").ap()
    o_dram = nc.dram_tensor("out", [HPC, S, D], F32, kind="ExternalOutput").ap()

    with (
        tc.tile_pool(name="const", bufs=1) as const_pool,
        tc.tile_pool(name="head_io", bufs=2) as head_pool,
        tc.tile_pool(name="qkt", bufs=2) as qkt_pool,
        tc.tile_pool(name="et", bufs=4) as et_pool,
        tc.tile_pool(name="epi", bufs=2) as epi_pool,
        tc.tile_pool(name="ps_s", bufs=2, space="PSUM") as ps_s_pool,
        tc.tile_pool(name="ps_o", bufs=1, space="PSUM") as ps_o_pool,
        tc.tile_pool(name="ps_t", bufs=2, space="PSUM") as ps_t_pool,
    ):
        ident16 = const_pool.tile([128, 128], F16)
        make_identity(nc, ident16[:])
        ones = const_pool.tile([128, NQ], F16)
        nc.vector.memset(ones[:], 1.0)

        ctx = {
            "nc": nc, "q_dram": q_dram, "k_dram": k_dram, "v_dram": v_dram,
            "head_pool": head_pool, "qkt_pool": qkt_pool, "ps_t_pool": ps_t_pool,
            "ident16": ident16, "ones": ones,
        }

        heads = [_HeadInputs(ctx, h) for h in range(HPC)]
        heads[0].start_dma()

        def emit_scores(hd, qh, k, s_ps):
            for b in range(NB):
                q0 = qh * QH + b * 512
                nc.tensor.matmul(
                    s_ps[:, b * 512:(b + 1) * 512],
                    hd.kT[:, k * 128:(k + 1) * 128],
                    hd.qT[:, q0:q0 + 512],
                    start=True, stop=True,
                )

        def emit_pv(hd, oT, k, et):
            for b in range(NB):
                nc.tensor.matmul(
                    oT[:, b * 512:(b + 1) * 512],
                    hd.v1[:, k, :],
                    et[:, b * 512:(b + 1) * 512],
                    start=(k == 0), stop=(k == NQ - 1),
                )

        def emit_epilogue(h, qh, oT):
            """Transpose oT back to [q, d], normalize, DMA out. Batched in
            groups of 4 q-blocks to avoid PE<->DVE ping-pong."""
            oT16 = epi_pool.tile([D + 1, QH], F16, tag="oT16", name="oT16")
            nc.vector.tensor_copy(oT16[:], oT[:])
            for g in range(QH // 512):  # 2 groups of 4 q-blocks
                # inner dim padded to 66 so each [:, j, :] slice is 4B-aligned
                tr = ps_t_pool.tile([128, 4, D + 2], F16, tag="t", name="tr")
                for j in range(4):
                    qb = g * 4 + j
                    nc.tensor.transpose(
                        tr[:, j, :D + 1], oT16[:, qb * 128:(qb + 1) * 128],
                        ctx["ident16"][:D + 1, :D + 1],
                    )
                den = epi_pool.tile([128, 4], F32, tag="den", name="den")
                nc.vector.tensor_scalar_add(den[:], tr[:, :, D], 1.0)
                rec = epi_pool.tile([128, 4], F32, tag="rec", name="rec")
                nc.vector.reciprocal(rec[:], den[:])
                o_sb = epi_pool.tile([128, 4, D], F32, tag="o_sb", name="o_sb")
                for j in range(4):
                    nc.vector.tensor_scalar_mul(o_sb[:, j, :], tr[:, j, :D], rec[:, j:j + 1])
                og = g * 4
                nc.sync.dma_start(
                    o_dram[h].rearrange("(n p) d -> p n d", p=128)[:, qh * 8 + og:qh * 8 + og + 4, :],
                    o_sb[:],
                )

        pending_epi = []
        for h in range(HPC):
            hd = heads[h]
            for qh in range(S // QH):
                # prefetch the next head's staging during the second pass
                if qh == 1 and h + 1 < HPC:
                    heads[h + 1].start_dma()

                oT = ps_o_pool.tile([D + 1, QH], F32, tag="oT", name="oT")
                s_tiles = {}
                s_tiles[0] = ps_s_pool.tile([128, QH], F32, tag="s", name="s0")
                emit_scores(hd, qh, 0, s_tiles[0])
                et_tiles = {}
                for k in range(NQ):
                    et_tiles[k] = et_pool.tile([128, QH], F16, tag="et", name=f"et{k}")
                    nc.scalar.activation(et_tiles[k][:], s_tiles[k][:], EXP, scale=SCALE)
                    if k + 1 < NQ:
                        s_tiles[k + 1] = ps_s_pool.tile([128, QH], F32, tag="s", name=f"s{k + 1}")
                        emit_scores(hd, qh, k + 1, s_tiles[k + 1])
                    # drain a previous pass's epilogue into the PE slack
                    if pending_epi and k == 2:
                        emit_epilogue(*pending_epi.pop(0))
                    emit_pv(hd, oT, k, et_tiles[k])
                    del et_tiles[k], s_tiles[k]
                pending_epi.append((h, qh, oT))
        while pending_epi:
            emit_epilogue(*pending_epi.pop(0))


_NC_CACHE = None
_TRACE_READY = False


def _enable_tracing():
    """Register the NTFF profile hook that this image's antenv lacks, and
    keep profiling artifacts local instead of uploading to a bucket."""
    global _TRACE_READY
    if _TRACE_READY:
        return
    import sys
    import types

    import antenv
    import concourse.bass_utils as bu
    from trn_agent_boot.trn_boot import _ntff_profile_via_ctypes

    if "antenv.axon_hooks" not in sys.modules:
        mod = types.ModuleType("antenv.axon_hooks")
        mod._hook = None

        def set_axon_ntff_profile_hook(h):
            mod._hook = h

        def get_axon_ntff_profile_hook():
            return mod._hook

        mod.set_axon_ntff_profile_hook = set_axon_ntff_profile_hook
        mod.get_axon_ntff_profile_hook = get_axon_ntff_profile_hook
        sys.modules["antenv.axon_hooks"] = mod
        antenv.axon_hooks = mod

    hooks = sys.modules["antenv.axon_hooks"]
    if hooks.get_axon_ntff_profile_hook() is None:
        hooks.set_axon_ntff_profile_hook(
            _ntff_profile_via_ctypes("/opt/axon/libaxon_pjrt.so")
        )
    bu.upload_artifacts = lambda tmpdir: tmpdir
    _TRACE_READY = True


def _build():
    global _NC_CACHE
    if _NC_CACHE is None:
        nc = bacc.Bacc("TRN2", target_bir_lowering=False, debug=False)
        with tile.TileContext(nc) as tc:
            _attention(tc)
        nc.compile()
        _NC_CACHE = nc
    return _NC_CACHE


def _run(query, key, value, trace=False, tmpdir=None):
    if trace:
        _enable_tracing()
    q = np.ascontiguousarray(np.asarray(query, dtype=np.float32).reshape(B * H, S, D))
    k = np.ascontiguousarray(np.asarray(key, dtype=np.float32).reshape(B * H, S, D))
    v = np.ascontiguousarray(np.asarray(value, dtype=np.float32).reshape(B * H, S, D))
    in_maps = [
        {
            "query": q[c * HPC:(c + 1) * HPC],
            "key": k[c * HPC:(c + 1) * HPC],
            "value": v[c * HPC:(c + 1) * HPC],
        }
        for c in range(N_CORES)
    ]
    nc = _build()
    res = run_bass_kernel_spmd(
        nc, in_maps, core_ids=list(range(N_CORES)), trace=trace, tmpdir=tmpdir
    )
    out = np.stack([res.results[c]["out"] for c in range(N_CORES)])  # [8, HPC, S, D]
    return out.reshape(B, H, S, D), res


def kernel(query, key, value):
    out, _ = _run(query, key, value, trace=bool(int(os.environ.get("BASS_TRACE", "0"))))
    return out


# revision 17
# speedup vs baseline: 1.5195x; 1.5195x over previous
"""Multi-head attention with "restricted softmax" on 8 TRN2 NeuronCores.

Reference computation (per head):
    score = Q @ K.T / sqrt(D)                       # [S, S]
    attn  = exp(score) / (1 + sum_k exp(score))     # restricted softmax
            (mathematically identical to the max-clamped reference form)
    out   = attn @ V                                # [S, D]

Full problem: B=2, H=16, S=2048, D=64  ->  32 heads, 4 heads per core.

Per-core kernel strategy (no communication needed):
  - Scores computed TRANSPOSED (S^T[k, q]) so softmax's k-reduction sits on
    the PSUM partition axis where the PE performs it for free: PV uses
    lhsT=[V | 1] so the extra output row is sum_k exp = the denominator.
  - All matmul operands are fp16 (PE streams 1 col/cycle vs fp32's slower
    modes) with fp32 PSUM accumulation.
  - The scores contraction (d=64) is ZERO-PADDED to K=128: half-height
    weights block LDWEIGHTS pipelining (measured 427 ns vs 216 ns per
    512-col matmul); zeros in kT rows 64-127 make qT's bottom half inert.
  - exp is fused with the 1/sqrt(D) scale on the ScalarEngine while
    evicting scores PSUM->SBUF (the ScalarEngine is the co-bottleneck).
  - Emission is software-pipelined: scores(k+1) before PV(k); transpose
    prologues are spread into earlier passes' PE slack.
"""

import os

import numpy as np

import concourse.bass as bass  # noqa: F401  (bass must import before tile)
import concourse.mybir as mybir
import concourse.tile as tile
from concourse import bacc
from concourse.bass_utils import run_bass_kernel_spmd
from concourse.masks import make_identity

B, H, S, D = 2, 16, 2048, 64
N_CORES = 8
HPC = (B * H) // N_CORES  # heads per core = 4

F32 = mybir.dt.float32
F16 = mybir.dt.float16
EXP = mybir.ActivationFunctionType.Exp

SCALE = 1.0 / 8.0  # 1/sqrt(D)
NQ = S // 128      # 16 tiles of 128 along both q and k
QH = 1024          # q-half width processed per pass
NB = QH // 512     # 512-wide matmuls per scores tile


class _HeadInputs:
    """Per-head staged inputs: fp16 Q^T/K^T [128, S] (rows 0..63 data,
    rows 64..127 exact zeros so the scores matmul contracts over K=128 —
    half-height weights block LDWEIGHTS pipelining) and [V|1].

    The transposes run on the DMA X-bar (fp16, [128,128] tiles): zero PE
    cost, and the d-padding produces the zero rows for free."""

    def __init__(self, ctx, h):
        self.ctx = ctx
        self.h = h

    def start_dma(self):
        nc, pools, h = self.ctx["nc"], self.ctx, self.h
        head_pool = pools["head_pool"]
        q_nat = head_pool.tile([128, NQ, D], F32, tag="q_nat", name=f"q_nat{h}")
        nc.sync.dma_start(q_nat[:], pools["q_dram"][h].rearrange("(n p) d -> p n d", p=128))
        k_nat = head_pool.tile([128, NQ, D], F32, tag="k_nat", name=f"k_nat{h}")
        nc.sync.dma_start(k_nat[:], pools["k_dram"][h].rearrange("(n p) d -> p n d", p=128))
        v_nat = head_pool.tile([128, NQ, D], F32, tag="v_nat", name=f"v_nat{h}")
        nc.sync.dma_start(v_nat[:], pools["v_dram"][h].rearrange("(n p) d -> p n d", p=128))

        # fp16 staging with d padded 64->128; pad columns stay zero across
        # slot reuse (each head only rewrites cols 0..63), cleared on the
        # first two uses (one per pool slot).
        q16 = head_pool.tile([128, NQ, 128], F16, tag="q16", name=f"q16_{h}")
        k16 = head_pool.tile([128, NQ, 128], F16, tag="k16", name=f"k16_{h}")
        if h < 2:
            nc.vector.memset(q16[:, :, D:], 0.0)
            nc.vector.memset(k16[:, :, D:], 0.0)
        nc.vector.tensor_copy(q16[:, :, :D], q_nat[:])
        nc.vector.tensor_copy(k16[:, :, :D], k_nat[:])
        v1 = head_pool.tile([128, NQ, D + 1], F16, tag="v1", name=f"v1_{h}")
        nc.vector.tensor_copy(
            v1[:, :, D:].rearrange("p n one -> p (n one)"), pools["ones"][:]
        )
        nc.vector.tensor_copy(v1[:, :, :D], v_nat[:])
        self.q16, self.k16, self.v1 = q16, k16, v1

        # bounce the padded fp16 staging through DRAM so the X-bar transpose
        # is ONE [2048, 128] -> [128, 2048] DMA per tensor (32KB-granular
        # SBUF-side transposes are overhead-bound: measured 1.24us each)
        dram_pool = pools["dram_pool"]
        qdr = dram_pool.tile([S, 128], F16, tag="qdr", name=f"qdr{h}")
        nc.sync.dma_start(qdr[:].rearrange("(n p) c -> p n c", p=128), self.q16[:])
        kdr = dram_pool.tile([S, 128], F16, tag="kdr", name=f"kdr{h}")
        nc.sync.dma_start(kdr[:].rearrange("(n p) c -> p n c", p=128), self.k16[:])

        qkt_pool = pools["qkt_pool"]
        self.qT = qkt_pool.tile([128, S], F16, tag="qT", name=f"qT{h}")
        self.kT = qkt_pool.tile([128, S], F16, tag="kT", name=f"kT{h}")
        nc.sync.dma_start_transpose(self.qT[:], qdr[:])
        nc.sync.dma_start_transpose(self.kT[:], kdr[:])


def _attention(tc):
    nc = tc.nc
    q_dram = nc.dram_tensor("query", [HPC, S, D], F32, kind="ExternalInput").ap()
    k_dram = nc.dram_tensor("key", [HPC, S, D], F32, kind="ExternalInput").ap()
    v_dram = nc.dram_tensor("value", [HPC, S, D], F32, kind="ExternalInput").ap()
    o_dram = nc.dram_tensor("out", [HPC, S, D], F32, kind="ExternalOutput").ap()

    with (
        tc.tile_pool(name="const", bufs=1) as const_pool,
        tc.tile_pool(name="head_io", bufs=2) as head_pool,
        tc.tile_pool(name="qkt", bufs=2) as qkt_pool,
        tc.tile_pool(name="et", bufs=4) as et_pool,
        tc.tile_pool(name="epi", bufs=2) as epi_pool,
        tc.tile_pool(name="dram", bufs=2, space="DRAM") as dram_pool,
        tc.tile_pool(name="ps_s", bufs=2, space="PSUM") as ps_s_pool,
        tc.tile_pool(name="ps_o", bufs=1, space="PSUM") as ps_o_pool,
        tc.tile_pool(name="ps_t", bufs=2, space="PSUM") as ps_t_pool,
    ):
        ident16 = const_pool.tile([128, 128], F16)
        make_identity(nc, ident16[:])
        ones = const_pool.tile([128, NQ], F16)
        nc.vector.memset(ones[:], 1.0)

        ctx = {
            "nc": nc, "q_dram": q_dram, "k_dram": k_dram, "v_dram": v_dram,
            "head_pool": head_pool, "qkt_pool": qkt_pool, "ps_t_pool": ps_t_pool,
            "dram_pool": dram_pool,
            "ident16": ident16, "ones": ones,
        }

        heads = [_HeadInputs(ctx, h) for h in range(HPC)]
        heads[0].start_dma()

        def emit_scores(hd, qh, k, s_ps):
            for b in range(NB):
                q0 = qh * QH + b * 512
                nc.tensor.matmul(
                    s_ps[:, b * 512:(b + 1) * 512],
                    hd.kT[:, k * 128:(k + 1) * 128],
                    hd.qT[:, q0:q0 + 512],
                    start=True, stop=True,
                )

        def emit_pv(hd, oT, k, et):
            for b in range(NB):
                nc.tensor.matmul(
                    oT[:, b * 512:(b + 1) * 512],
                    hd.v1[:, k, :],
                    et[:, b * 512:(b + 1) * 512],
                    start=(k == 0), stop=(k == NQ - 1),
                )

        def emit_epilogue(h, qh, oT):
            """Transpose oT back to [q, d], normalize, DMA out. Batched in
            groups of 4 q-blocks to avoid PE<->DVE ping-pong."""
            oT16 = epi_pool.tile([D + 1, QH], F16, tag="oT16", name="oT16")
            nc.vector.tensor_copy(oT16[:], oT[:])
            for g in range(QH // 512):  # 2 groups of 4 q-blocks
                # inner dim padded to 66 so each [:, j, :] slice is 4B-aligned
                tr = ps_t_pool.tile([128, 4, D + 2], F16, tag="t", name="tr")
                for j in range(4):
                    qb = g * 4 + j
                    nc.tensor.transpose(
                        tr[:, j, :D + 1], oT16[:, qb * 128:(qb + 1) * 128],
                        ctx["ident16"][:D + 1, :D + 1],
                    )
                den = epi_pool.tile([128, 4], F32, tag="den", name="den")
                nc.vector.tensor_scalar_add(den[:], tr[:, :, D], 1.0)
                rec = epi_pool.tile([128, 4], F32, tag="rec", name="rec")
                nc.vector.reciprocal(rec[:], den[:])
                o_sb = epi_pool.tile([128, 4, D], F32, tag="o_sb", name="o_sb")
                for j in range(4):
                    nc.vector.tensor_scalar_mul(o_sb[:, j, :], tr[:, j, :D], rec[:, j:j + 1])
                og = g * 4
                nc.sync.dma_start(
                    o_dram[h].rearrange("(n p) d -> p n d", p=128)[:, qh * 8 + og:qh * 8 + og + 4, :],
                    o_sb[:],
                )

        pending_epi = []
        for h in range(HPC):
            hd = heads[h]
            for qh in range(S // QH):
                # prefetch the next head's staging during the second pass
                if qh == 1 and h + 1 < HPC:
                    heads[h + 1].start_dma()

                oT = ps_o_pool.tile([D + 1, QH], F32, tag="oT", name="oT")
                s_tiles = {}
                s_tiles[0] = ps_s_pool.tile([128, QH], F32, tag="s", name="s0")
                emit_scores(hd, qh, 0, s_tiles[0])
                et_tiles = {}
                for k in range(NQ):
                    et_tiles[k] = et_pool.tile([128, QH], F16, tag="et", name=f"et{k}")
                    nc.scalar.activation(et_tiles[k][:], s_tiles[k][:], EXP, scale=SCALE)
                    if k + 1 < NQ:
                        s_tiles[k + 1] = ps_s_pool.tile([128, QH], F32, tag="s", name=f"s{k + 1}")
                        emit_scores(hd, qh, k + 1, s_tiles[k + 1])
                    # drain a previous pass's epilogue into the PE slack
                    if pending_epi and k == 2:
                        emit_epilogue(*pending_epi.pop(0))
                    emit_pv(hd, oT, k, et_tiles[k])
                    del et_tiles[k], s_tiles[k]
                pending_epi.append((h, qh, oT))
        while pending_epi:
            emit_epilogue(*pending_epi.pop(0))


_NC_CACHE = None
_TRACE_READY = False


def _enable_tracing():
    """Register the NTFF profile hook that this image's antenv lacks, and
    keep profiling artifacts local instead of uploading to a bucket."""
    global _TRACE_READY
    if _TRACE_READY:
        return
    import sys
    import types

    import antenv
    import concourse.bass_utils as bu
    from trn_agent_boot.trn_boot import _ntff_profile_via_ctypes

    if "antenv.axon_hooks" not in sys.modules:
        mod = types.ModuleType("antenv.axon_hooks")
        mod._hook = None

        def set_axon_ntff_profile_hook(h):
            mod._hook = h

        def get_axon_ntff_profile_hook():
            return mod._hook

        mod.set_axon_ntff_profile_hook = set_axon_ntff_profile_hook
        mod.get_axon_ntff_profile_hook = get_axon_ntff_profile_hook
        sys.modules["antenv.axon_hooks"] = mod
        antenv.axon_hooks = mod

    hooks = sys.modules["antenv.axon_hooks"]
    if hooks.get_axon_ntff_profile_hook() is None:
        hooks.set_axon_ntff_profile_hook(
            _ntff_profile_via_ctypes("/opt/axon/libaxon_pjrt.so")
        )
    bu.upload_artifacts = lambda tmpdir: tmpdir
    _TRACE_READY = True


def _build():
    global _NC_CACHE
    if _NC_CACHE is None:
        nc = bacc.Bacc("TRN2", target_bir_lowering=False, debug=False)
        with tile.TileContext(nc) as tc:
            _attention(tc)
        nc.compile()
        _NC_CACHE = nc
    return _NC_CACHE


def _run(query, key, value, trace=False, tmpdir=None):
    if trace:
        _enable_tracing()
    q = np.ascontiguousarray(np.asarray(query, dtype=np.float32).reshape(B * H, S, D))
    k = np.ascontiguousarray(np.asarray(key, dtype=np.float32).reshape(B * H, S, D))
    v = np.ascontiguousarray(np.asarray(value, dtype=np.float32).reshape(B * H, S, D))
    in_maps = [
        {
            "query": q[c * HPC:(c + 1) * HPC],
            "key": k[c * HPC:(c + 1) * HPC],
            "value": v[c * HPC:(c + 1) * HPC],
        }
        for c in range(N_CORES)
    ]
    nc = _build()
    res = run_bass_kernel_spmd(
        nc, in_maps, core_ids=list(range(N_CORES)), trace=trace, tmpdir=tmpdir
    )
    out = np.stack([res.results[c]["out"] for c in range(N_CORES)])  # [8, HPC, S, D]
    return out.reshape(B, H, S, D), res


def kernel(query, key, value):
    out, _ = _run(query, key, value, trace=bool(int(os.environ.get("BASS_TRACE", "0"))))
    return out
